# revision 11
# baseline (speedup 1.0000x reference)
"""Trainium2 Bass kernel for nn_EpisodicMemory (retrieval_knn).

Pipeline (4 SPMD launches, all compiled once per process and cached):
  A  (8 cores): episode-bank scoring. Each core owns 128 episodes [128,128,512]
     loaded episode-major ([128 ep-partitions, 16 L, 512 D] tiles, 32KB
     contiguous per partition -> line-rate DMA descriptors). The L-sum is a
     DVE in-place tree-add per tile (hidden under the DMA), then one
     mul+reduce against the host-precomputed v = Wk.T @ (Wq q + bq) / L.
  host: stable top-k, recency prescale of the 5 selected episodes.
  B1 (2 cores): biLSTM layer 0, transposed layout (gate dim -> partitions,
     batch -> free). Per step: 2 pre-gate injection matmuls (identity x preT,
     emitted one step ahead so they fill the PE during the previous step's
     elementwise tail), 16 recurrent matmuls (g-gates first so tanh(g) runs
     under the i/f/o matmuls), one sigmoid over [i|f|o], fused [z|w] multiply
     against the SBUF-resident [tg|c] state, add, tanh, and a bf16 multiply
     that writes h directly into the time-major history buffer.
  B2 (2 cores): biLSTM layer 1 -- same compiled program; the host repacks the
     layer-0 histories (transpose + time flip) between launches.
  B3 (1 core): temporal attention over the 5 scanned episodes in fp32.
"""

import numpy as np
import ml_dtypes

BF16 = ml_dtypes.bfloat16

N, L, D, H = 1024, 128, 512, 256
K = 5
NC = 8
EPC = N // NC  # 128 episodes per core
G4 = 4 * H     # 1024 gate dims
NGC = G4 // 128  # 8 gate chunks
NHC = H // 128   # 2 hidden chunks
BK = NHC * K     # 10

_cache = {}


def _enable_fwl():
    # concourse pins --enable-ldw-opt=false; walrus rejects bass InstLdweights
    # under ldw-opt, so leave it off (FWL still engages for bf16 128-col
    # stationaries per the compiler's automatic EnableFWL path).
    pass


# --------------------------------------------------------------------------
# program builders
# --------------------------------------------------------------------------

def build_phase_a():
    import concourse.bacc as bacc
    import concourse.mybir as mybir
    from concourse.tile import TileContext

    dt = mybir.dt
    f32 = dt.float32
    nc = bacc.Bacc("TRN2", target_bir_lowering=False, debug=False, num_devices=NC)
    ep = nc.dram_tensor("ep", [EPC, L, D], f32, kind="ExternalInput")
    vrep = nc.dram_tensor("vrep", [128, D], f32, kind="ExternalInput")
    scores = nc.dram_tensor("scores", [EPC, 1], f32, kind="ExternalOutput")

    LC = 8  # L rows per DMA tile: [128, 8, 512] f32 = 16KB/partition contiguous

    from contextlib import ExitStack
    with TileContext(nc) as tc, ExitStack() as ectx:
        const = ectx.enter_context(tc.tile_pool(name="const", bufs=1))
        dma_p = ectx.enter_context(tc.tile_pool(name="eps", bufs=10))

        # all episode DMAs on ONE HWDGE ring: the SDMA engines drain the ring
        # FIFO, so tile 0 completes first instead of all tiles progressing in
        # lockstep (which delays the first reduction by ~20us). Deep buffering
        # (10 tiles) decouples the DMA stream from the DVE reduction.
        tiles = []
        for i in range(L // LC):
            t = dma_p.tile([128, LC, D], f32, tag="ep")
            nc.sync.dma_start(out=t, in_=ep[:, LC * i:LC * (i + 1), :])
            tiles.append(t)
        vrep_sb = const.tile([128, D], f32)
        nc.scalar.dma_start(out=vrep_sb, in_=vrep[:, :])
        acc = const.tile([128, D], f32)
        nc.vector.memset(acc, 0.0)

        for i in range(L // LC):
            t = tiles[i]
            # in-place tree reduction over the 8 L-slabs
            nc.vector.tensor_add(t[:, 0:4, :], t[:, 0:4, :], t[:, 4:8, :])
            nc.vector.tensor_add(t[:, 0:2, :], t[:, 0:2, :], t[:, 2:4, :])
            nc.vector.tensor_add(t[:, 0, :], t[:, 0, :], t[:, 1, :])
            nc.vector.tensor_add(acc, acc, t[:, 0, :])

        scr = const.tile([128, D], f32)
        nc.vector.tensor_mul(scr, acc, vrep_sb)
        ssb = const.tile([128, 1], f32)
        nc.vector.tensor_reduce(ssb, scr, axis=mybir.AxisListType.X,
                                op=mybir.AluOpType.add)
        nc.sync.dma_start(out=scores[:, :], in_=ssb)
    nc.compile()
    return nc


def build_phase_b():
    """One biLSTM layer step (used for both layers; SPMD over 2 cores =
    fwd/bwd direction). Input x is the host-packed [128, 4, K, L] bf16
    transposed layout; output is the time-major bf16 history [128, L, BK]."""
    import concourse.bacc as bacc
    import concourse.mybir as mybir
    from concourse.tile import TileContext

    dt = mybir.dt
    AF = mybir.ActivationFunctionType
    f32, bf = dt.float32, dt.bfloat16
    nc = bacc.Bacc("TRN2", target_bir_lowering=False, debug=False, num_devices=2)
    kc_in = 4

    wih = nc.dram_tensor("wih", [D, G4], bf, kind="ExternalInput")
    whh = nc.dram_tensor("whh", [H, G4], bf, kind="ExternalInput")
    bias = nc.dram_tensor("bias", [G4], f32, kind="ExternalInput")
    x = nc.dram_tensor("x", [128, kc_in, K, L], bf, kind="ExternalInput")
    hout = nc.dram_tensor("hout", [128, L, BK], bf, kind="ExternalOutput")

    id_bf = nc.inline_tensor(np.eye(128, dtype=BF16), "idbf")

    from contextlib import ExitStack
    with TileContext(nc) as tc, ExitStack() as ectx:
        const = ectx.enter_context(tc.tile_pool(name="const", bufs=1))

        # warm the ACT tables (sigmoid/tanh ~2.6us each) under the weight DMAs
        warm = const.tile([128, 1], f32)
        nc.vector.memset(warm, 0.0)
        nc.scalar.activation(warm, warm, AF.Sigmoid)
        nc.scalar.activation(warm, warm, AF.Tanh)

        ident_bf = const.tile([128, 128], bf)
        nc.gpsimd.dma_start(out=ident_bf, in_=id_bf[:, :])
        wih_sb = const.tile([128, kc_in, G4], bf)
        nc.sync.dma_start(out=wih_sb[:, 0:2, :],
                          in_=wih.rearrange("(kc p) g -> p kc g", p=128)[:, 0:2, :])
        nc.scalar.dma_start(out=wih_sb[:, 2:4, :],
                            in_=wih.rearrange("(kc p) g -> p kc g", p=128)[:, 2:4, :])
        whh_sb = const.tile([128, NHC, G4], bf)
        nc.gpsimd.dma_start(out=whh_sb, in_=whh.rearrange("(hc p) g -> p hc g", p=128))
        bias_sb = const.tile([128, NGC], f32)
        nc.gpsimd.dma_start(out=bias_sb, in_=bias.rearrange("(gc p) -> p gc", p=128))
        xT = const.tile([128, kc_in, K, L], bf)
        nc.sync.dma_start(out=xT[:, 0:2, :, :], in_=x[:, 0:2, :, :])
        nc.scalar.dma_start(out=xT[:, 2:4, :, :], in_=x[:, 2:4, :, :])

        # warm the PE HAM clock gate (1.2 -> 2.4 GHz takes ~3.4us of sustained
        # activity) with dummy matmuls while the weight DMAs land, so the
        # pre-projection and the first scan steps run at full clock.
        with ExitStack() as wctx:
            warm_ps = wctx.enter_context(tc.tile_pool(name="warm_ps", bufs=1,
                                                      space="PSUM"))
            wps = warm_ps.tile([128, 128], f32)
            for _ in range(56):
                nc.tensor.matmul(wps, ident_bf, ident_bf, start=True, stop=True)

        # ---- pre-projection: preT[:, gc, :, :] = wih[:, gc-cols].T @ xT + b
        preT = const.tile([128, NGC, K, L], bf)
        with ExitStack() as pctx:
            pre_ps = pctx.enter_context(tc.tile_pool(name="pre_ps", bufs=2,
                                                     space="PSUM"))
            for gc in range(NGC):
                psA = pre_ps.tile([128, 512], f32, tag="preA")
                psB = pre_ps.tile([128, 128], f32, tag="preB")
                for kc in range(kc_in):
                    lhsT = wih_sb[:, kc, 128 * gc:128 * (gc + 1)]
                    nc.tensor.matmul(psA, lhsT, xT[:, kc, 0:4, :],
                                     start=(kc == 0), stop=(kc == kc_in - 1))
                    nc.tensor.matmul(psB, lhsT, xT[:, kc, 4, :],
                                     start=(kc == 0), stop=(kc == kc_in - 1))
                bb = bias_sb[:, gc:gc + 1]
                nc.vector.tensor_add(preT[:, gc, 0:4, :], psA,
                                     bb.to_broadcast([128, 512]))
                nc.vector.tensor_add(preT[:, gc, 4, :], psB,
                                     bb.to_broadcast([128, 128]))

        # ---- 128-step scan
        step_ps = ectx.enter_context(tc.tile_pool(name="step_ps", bufs=3,
                                                  space="PSUM"))
        step_sb = ectx.enter_context(tc.tile_pool(name="step_sb", bufs=3))

        hbufT = const.tile([128, L + 1, BK], bf)   # time-major history
        tc_st = const.tile([128, 2 * BK], f32)     # [tg | c] cell state
        nc.vector.memset(hbufT[:, 0, :], 0.0)
        nc.vector.memset(tc_st[:, BK:2 * BK], 0.0)

        for t in range(L):
            ps_ifo = step_ps.tile([128, 6 * K], f32, tag="ifo", bufs=3)
            ps_g = step_ps.tile([128, 2 * K], f32, tag="g", bufs=3)
            # pre-gate injection: no dependence on h, so the PE executes these
            # during the previous step's elementwise tail.
            nc.tensor.matmul(ps_ifo, ident_bf, preT[:, 0:6, :, t],
                             start=True, stop=False)
            nc.tensor.matmul(ps_g, ident_bf, preT[:, 6:8, :, t],
                             start=True, stop=False)

            def h_rhs(hc):
                return hbufT[:, t, K * hc:K * (hc + 1)]

            # matmul order: g (tanh runs under the rest), then i/f (the
            # sigmoid the chain waits on), then o (its sigmoid is off-chain).
            for gc in (6, 7):
                for hc in range(NHC):
                    nc.tensor.matmul(
                        ps_g[:, K * (gc - 6):K * (gc - 5)],
                        whh_sb[:, hc, 128 * gc:128 * (gc + 1)], h_rhs(hc),
                        start=False, stop=(gc == 7 and hc == NHC - 1))
            for gc in (0, 1, 2, 3):
                for hc in range(NHC):
                    nc.tensor.matmul(
                        ps_ifo[:, K * gc:K * (gc + 1)],
                        whh_sb[:, hc, 128 * gc:128 * (gc + 1)], h_rhs(hc),
                        start=False, stop=(gc == 3 and hc == NHC - 1))
            for gc in (4, 5):
                for hc in range(NHC):
                    nc.tensor.matmul(
                        ps_ifo[:, K * gc:K * (gc + 1)],
                        whh_sb[:, hc, 128 * gc:128 * (gc + 1)], h_rhs(hc),
                        start=False, stop=(gc == 5 and hc == NHC - 1))

            # tanh(g) lands in tc_st[0:BK] while the i/f/o matmuls still run
            nc.scalar.activation(tc_st[:, 0:BK], ps_g, AF.Tanh)
            sg = step_sb.tile([128, 3 * BK], f32, tag="sg", bufs=3)
            nc.scalar.activation(sg[:, 0:2 * BK], ps_ifo[:, 0:2 * BK], AF.Sigmoid)
            nc.scalar.activation(sg[:, 2 * BK:3 * BK], ps_ifo[:, 2 * BK:3 * BK],
                                 AF.Sigmoid)
            zw = step_sb.tile([128, 2 * BK], f32, tag="zw", bufs=3)
            nc.vector.tensor_mul(zw, sg[:, 0:2 * BK], tc_st)   # [si*tg | sf*c]
            nc.vector.tensor_add(tc_st[:, BK:2 * BK], zw[:, 0:BK],
                                 zw[:, BK:2 * BK])             # c'
            th = step_sb.tile([128, BK], f32, tag="th", bufs=3)
            nc.scalar.activation(th, tc_st[:, BK:2 * BK], AF.Tanh)
            nc.vector.tensor_mul(hbufT[:, t + 1, :], sg[:, 2 * BK:3 * BK], th)

        # stream the history out in quarters so only the last 32 steps' DMA
        # trails the scan
        for q in range(4):
            nc.sync.dma_start(out=hout[:, 32 * q:32 * (q + 1), :],
                              in_=hbufT[:, 1 + 32 * q:1 + 32 * (q + 1), :])
    nc.compile()
    return nc


def build_phase_b3():
    import concourse.bacc as bacc
    import concourse.mybir as mybir
    from concourse.tile import TileContext

    dt = mybir.dt
    AO = mybir.AluOpType
    AF = mybir.ActivationFunctionType
    f32 = dt.float32
    nc = bacc.Bacc("TRN2", target_bir_lowering=False, debug=False, num_devices=1)

    bf = dt.bfloat16
    h1f = nc.dram_tensor("h1f", [128, NHC, K, L], bf, kind="ExternalInput")
    h1b = nc.dram_tensor("h1b", [128, NHC, K, L], bf, kind="ExternalInput")
    cs = nc.dram_tensor("cs", [D], f32, kind="ExternalInput")
    ctx_out = nc.dram_tensor("ctx", [K, D], f32, kind="ExternalOutput")
    id_f32 = nc.inline_tensor(np.eye(128, dtype=np.float32), "idf")

    DC = D // 128  # 4 chunks
    from contextlib import ExitStack
    with TileContext(nc) as tc, ExitStack() as ectx:
        pool = ectx.enter_context(tc.tile_pool(name="sb", bufs=1))
        ps_p = ectx.enter_context(tc.tile_pool(name="ps", bufs=2, space="PSUM"))
        sc_p = ectx.enter_context(tc.tile_pool(name="scratch", bufs=2))

        # warm the exp table under the input DMAs
        warm = pool.tile([128, 1], f32)
        nc.vector.memset(warm, 0.0)
        nc.scalar.activation(warm, warm, AF.Exp)

        lout = pool.tile([128, DC, K, L], bf)
        nc.sync.dma_start(out=lout[:, 0:NHC, :, :], in_=h1f[:, :, :, :])
        nc.scalar.dma_start(out=lout[:, NHC:DC, :, :], in_=h1b[:, :, :, :])
        cs_f = pool.tile([128, DC], f32)
        nc.gpsimd.dma_start(out=cs_f, in_=cs.rearrange("(kc p) -> p kc", p=128))
        cs_sb = pool.tile([128, DC], bf)
        nc.vector.tensor_copy(cs_sb, cs_f)
        ident_f = pool.tile([128, 128], f32)
        nc.gpsimd.dma_start(out=ident_f, in_=id_f32[:, :])

        # warm the PE clock gate while the input DMAs land
        with ExitStack() as wctx:
            warm_ps = wctx.enter_context(tc.tile_pool(name="warm_ps", bufs=1,
                                                      space="PSUM"))
            wps = warm_ps.tile([128, 128], f32)
            for _ in range(40):
                nc.tensor.matmul(wps, ident_f, ident_f, start=True, stop=True)

        # stationary = cs column broadcast to 128 identical columns -> every
        # out partition carries the same score row (free partition-broadcast)
        psA = ps_p.tile([128, 512], f32, tag="attA")
        psB = ps_p.tile([128, 128], f32, tag="attB")
        for kc in range(DC):
            csb_rep = cs_sb[:, kc:kc + 1].to_broadcast([128, 128])
            nc.tensor.matmul(psA, csb_rep, lout[:, kc, 0:4, :],
                             start=(kc == 0), stop=(kc == DC - 1))
            nc.tensor.matmul(psB, csb_rep, lout[:, kc, 4, :],
                             start=(kc == 0), stop=(kc == DC - 1))
        esb = pool.tile([128, K, L], f32)
        nc.scalar.activation(esb[:, 0:4, :], psA, AF.Exp)
        nc.scalar.activation(esb[:, 4, :], psB, AF.Exp)
        se = pool.tile([128, K], f32)
        nc.vector.tensor_reduce(se, esb, axis=mybir.AxisListType.X, op=AO.add)
        rse = pool.tile([128, K], f32)
        nc.vector.reciprocal(rse, se)
        attw = pool.tile([128, K, L], f32)
        nc.vector.tensor_mul(attw, esb, rse.unsqueeze(2).to_broadcast([128, K, L]))

        ctxT = pool.tile([128, DC, K], f32)
        wsc = sc_p.tile([128, DC, K, L], f32, tag="wsc")
        nc.vector.tensor_mul(wsc, lout,
                             attw.unsqueeze(1).to_broadcast([128, DC, K, L]))
        nc.vector.tensor_reduce(ctxT, wsc, axis=mybir.AxisListType.X, op=AO.add)
        csb = pool.tile([K, DC, 128], f32)
        for kc in range(DC):
            pst = ps_p.tile([K, 128], f32, tag="tp")
            nc.tensor.transpose(pst, ctxT[:, kc, :], ident_f)
            nc.vector.tensor_copy(csb[:, kc, :], pst)
        nc.sync.dma_start(out=ctx_out[:, :], in_=csb)
    nc.compile()
    return nc


# --------------------------------------------------------------------------
# host-side weight prep
# --------------------------------------------------------------------------

def _prep_lstm_weights(w_ih, w_hh, b_ih, b_hh, perm_input_halves=False):
    def reorder(m):
        # torch gate order [i, f, g, o] -> kernel order [i, f, o, g]
        i, f, g, o = np.split(m, 4, axis=0)
        return np.concatenate([i, f, o, g], axis=0)

    wihT = np.ascontiguousarray(reorder(np.asarray(w_ih, np.float32)).T)
    whhT = np.ascontiguousarray(reorder(np.asarray(w_hh, np.float32)).T)
    bias = reorder((np.asarray(b_ih, np.float32) + np.asarray(b_hh, np.float32))[:, None])[:, 0]
    if perm_input_halves:
        wihT = np.concatenate([wihT[H:2 * H], wihT[0:H]], axis=0)
    return (np.ascontiguousarray(wihT.astype(BF16)),
            np.ascontiguousarray(whhT.astype(BF16)),
            np.ascontiguousarray(bias.astype(np.float32)))


def _get(name, builder):
    if name not in _cache:
        _cache[name] = builder()
    return _cache[name]


def _ensure_ntff_hook():
    """The image's antenv lacks axon_hooks; synthesize it and register the
    ctypes NTFF profiling hook from trn_agent_boot so trace=True works."""
    import sys
    import types
    try:
        from antenv.axon_hooks import get_axon_ntff_profile_hook  # noqa: F401
        return
    except ImportError:
        pass
    import antenv
    mod = types.ModuleType("antenv.axon_hooks")
    mod._hook = None

    def set_axon_ntff_profile_hook(h):
        mod._hook = h

    def get_axon_ntff_profile_hook():
        return mod._hook

    mod.set_axon_ntff_profile_hook = set_axon_ntff_profile_hook
    mod.get_axon_ntff_profile_hook = get_axon_ntff_profile_hook
    sys.modules["antenv.axon_hooks"] = mod
    antenv.axon_hooks = mod
    try:
        from trn_agent_boot.trn_boot import _ntff_profile_via_ctypes
        hook = _ntff_profile_via_ctypes('/opt/axon/libaxon_pjrt.so')
        if hook is not None:
            mod._hook = hook
    except Exception:
        pass


def _run(nc, in_maps, core_ids, trace=False):
    from concourse.bass_utils import run_bass_kernel_spmd
    if trace:
        try:
            _ensure_ntff_hook()
            return run_bass_kernel_spmd(nc, in_maps, core_ids, trace=True)
        except Exception as e:
            print(f"trace run failed ({type(e).__name__}: {e}); retrying untraced")
    return run_bass_kernel_spmd(nc, in_maps, core_ids, trace=False)


# --------------------------------------------------------------------------
# main entry
# --------------------------------------------------------------------------

def _pack_xT_from_h(hT, flip=False):
    """[128, L, BK] bf16 scan history -> [128, NHC, K, L] input chunk."""
    a = np.asarray(hT)
    if flip:
        a = a[:, ::-1, :]
    return np.transpose(a.reshape(128, L, NHC, K), (0, 2, 3, 1))


def kernel(episodes, query, current_state, ages, Wq, bq, Wk, bk,
           w_ih_l0, w_hh_l0, b_ih_l0, b_hh_l0,
           w_ih_l0r, w_hh_l0r, b_ih_l0r, b_hh_l0r,
           w_ih_l1, w_hh_l1, b_ih_l1, b_hh_l1,
           w_ih_l1r, w_hh_l1r, b_ih_l1r, b_hh_l1r, k,
           _collect_times=None):
    episodes = np.asarray(episodes, np.float32)
    query = np.asarray(query, np.float32)
    current_state = np.asarray(current_state, np.float32)
    ages = np.asarray(ages, np.float32)
    assert int(k) == K

    times = _collect_times if _collect_times is not None else None
    trace = times is not None

    def note(res):
        if times is not None:
            times.append(res.exec_time_ns)

    # ---- phase A
    qp = np.asarray(Wq, np.float32) @ query + np.asarray(bq, np.float32)
    v = (np.asarray(Wk, np.float32).T @ qp) / np.float32(L)
    vrep = np.ascontiguousarray(np.broadcast_to(v, (128, D)), dtype=np.float32)
    nc_a = _get("A", build_phase_a)
    in_maps = [{"ep": episodes[c * EPC:(c + 1) * EPC], "vrep": vrep}
               for c in range(NC)]
    res = _run(nc_a, in_maps, list(range(NC)), trace)
    note(res)
    scores = np.concatenate([res.results[c]["scores"][:, 0] for c in range(NC)])

    idx = np.argsort(-scores, kind="stable")[:K]
    w_rec = (1.0 / (1.0 + ages[idx] * np.float32(0.01))).astype(np.float32)
    xsel = episodes[idx] * w_rec[:, None, None]

    # ---- phase B1 (layer 0)
    wi0, wh0, b0 = _prep_lstm_weights(w_ih_l0, w_hh_l0, b_ih_l0, b_hh_l0)
    wi0r, wh0r, b0r = _prep_lstm_weights(w_ih_l0r, w_hh_l0r, b_ih_l0r, b_hh_l0r)
    nc_b = _get("B", build_phase_b)

    def to_xT(xs):  # [5, 128, 512] f32 -> [128, 4, 5, 128] bf16
        xT = np.transpose(xs, (2, 0, 1)).reshape(4, 128, K, L)
        return np.ascontiguousarray(np.transpose(xT, (1, 0, 2, 3)).astype(BF16))

    in_maps = [
        {"x": to_xT(xsel), "wih": wi0, "whh": wh0, "bias": b0},
        {"x": to_xT(xsel[:, ::-1, :]), "wih": wi0r, "whh": wh0r, "bias": b0r},
    ]
    res = _run(nc_b, in_maps, [0, 1], trace)
    note(res)
    h0_c0 = np.asarray(res.results[0]["hout"])  # bf16 [128, L, BK]
    h0_c1 = np.asarray(res.results[1]["hout"])

    # ---- phase B2 (layer 1)
    wi1, wh1, b1 = _prep_lstm_weights(w_ih_l1, w_hh_l1, b_ih_l1, b_hh_l1)
    wi1r, wh1r, b1r = _prep_lstm_weights(w_ih_l1r, w_hh_l1r, b_ih_l1r, b_hh_l1r,
                                         perm_input_halves=True)
    x_c0 = np.ascontiguousarray(np.concatenate(
        [_pack_xT_from_h(h0_c0), _pack_xT_from_h(h0_c1, flip=True)], axis=1))
    x_c1 = np.ascontiguousarray(np.concatenate(
        [_pack_xT_from_h(h0_c1), _pack_xT_from_h(h0_c0, flip=True)], axis=1))
    in_maps = [
        {"x": x_c0, "wih": wi1, "whh": wh1, "bias": b1},
        {"x": x_c1, "wih": wi1r, "whh": wh1r, "bias": b1r},
    ]
    res = _run(nc_b, in_maps, [0, 1], trace)
    note(res)
    h1_c0 = np.asarray(res.results[0]["hout"])  # bf16 [128, L, BK]
    h1_c1 = np.asarray(res.results[1]["hout"])

    # ---- phase B3 (attention)
    nc_b3 = _get("B3", build_phase_b3)
    in_maps = [{"h1f": np.ascontiguousarray(_pack_xT_from_h(h1_c0)),
                "h1b": np.ascontiguousarray(_pack_xT_from_h(h1_c1, flip=True)),
                "cs": current_state}]
    res = _run(nc_b3, in_maps, [0], trace)
    note(res)
    ctx = np.asarray(res.results[0]["ctx"], np.float32)  # [5, 512]
    return ctx[:, None, :]


# revision 13
# speedup vs baseline: 1.1763x; 1.1763x over previous
"""Trainium2 Bass kernel for nn_EpisodicMemory (retrieval_knn).

Pipeline (4 SPMD launches, all compiled once per process and cached):
  A  (8 cores): episode-bank scoring. Each core owns 128 episodes [128,128,512]
     loaded episode-major ([128 ep-partitions, 16 L, 512 D] tiles, 32KB
     contiguous per partition -> line-rate DMA descriptors). The L-sum is a
     DVE in-place tree-add per tile (hidden under the DMA), then one
     mul+reduce against the host-precomputed v = Wk.T @ (Wq q + bq) / L.
  host: stable top-k, recency prescale of the 5 selected episodes.
  B1 (2 cores): biLSTM layer 0, transposed layout (gate dim -> partitions,
     batch -> free). Per step: 2 pre-gate injection matmuls (identity x preT,
     emitted one step ahead so they fill the PE during the previous step's
     elementwise tail), 16 recurrent matmuls (g-gates first so tanh(g) runs
     under the i/f/o matmuls), one sigmoid over [i|f|o], fused [z|w] multiply
     against the SBUF-resident [tg|c] state, add, tanh, and a bf16 multiply
     that writes h directly into the time-major history buffer.
  B2 (2 cores): biLSTM layer 1 -- same compiled program; the host repacks the
     layer-0 histories (transpose + time flip) between launches.
  B3 (1 core): temporal attention over the 5 scanned episodes in fp32.
"""

import numpy as np
import ml_dtypes

BF16 = ml_dtypes.bfloat16

N, L, D, H = 1024, 128, 512, 256
K = 5
NC = 8
EPC = N // NC  # 128 episodes per core
G4 = 4 * H     # 1024 gate dims
NGC = G4 // 128  # 8 gate chunks
NHC = H // 128   # 2 hidden chunks
BK = NHC * K     # 10

_cache = {}


def _enable_fwl():
    # concourse pins --enable-ldw-opt=false; walrus rejects bass InstLdweights
    # under ldw-opt, so leave it off (FWL still engages for bf16 128-col
    # stationaries per the compiler's automatic EnableFWL path).
    pass


# --------------------------------------------------------------------------
# program builders
# --------------------------------------------------------------------------

def build_phase_a():
    import concourse.bacc as bacc
    import concourse.mybir as mybir
    from concourse.tile import TileContext

    dt = mybir.dt
    f32 = dt.float32
    nc = bacc.Bacc("TRN2", target_bir_lowering=False, debug=False, num_devices=NC)
    ep = nc.dram_tensor("ep", [EPC, L, D], f32, kind="ExternalInput")
    vrep = nc.dram_tensor("vrep", [128, D], f32, kind="ExternalInput")
    scores = nc.dram_tensor("scores", [EPC, 1], f32, kind="ExternalOutput")

    LC = 8  # L rows per DMA tile: [128, 8, 512] f32 = 16KB/partition contiguous

    from contextlib import ExitStack
    with TileContext(nc) as tc, ExitStack() as ectx:
        const = ectx.enter_context(tc.tile_pool(name="const", bufs=1))
        dma_p = ectx.enter_context(tc.tile_pool(name="eps", bufs=10))

        # all episode DMAs on ONE HWDGE ring: the SDMA engines drain the ring
        # FIFO, so tile 0 completes first instead of all tiles progressing in
        # lockstep (which delays the first reduction by ~20us). Deep buffering
        # (10 tiles) decouples the DMA stream from the DVE reduction.
        tiles = []
        for i in range(L // LC):
            t = dma_p.tile([128, LC, D], f32, tag="ep")
            nc.sync.dma_start(out=t, in_=ep[:, LC * i:LC * (i + 1), :])
            tiles.append(t)
        vrep_sb = const.tile([128, D], f32)
        nc.scalar.dma_start(out=vrep_sb, in_=vrep[:, :])
        acc = const.tile([128, D], f32)
        nc.vector.memset(acc, 0.0)

        for i in range(L // LC):
            t = tiles[i]
            # in-place tree reduction over the 8 L-slabs
            nc.vector.tensor_add(t[:, 0:4, :], t[:, 0:4, :], t[:, 4:8, :])
            nc.vector.tensor_add(t[:, 0:2, :], t[:, 0:2, :], t[:, 2:4, :])
            nc.vector.tensor_add(t[:, 0, :], t[:, 0, :], t[:, 1, :])
            nc.vector.tensor_add(acc, acc, t[:, 0, :])

        scr = const.tile([128, D], f32)
        nc.vector.tensor_mul(scr, acc, vrep_sb)
        ssb = const.tile([128, 1], f32)
        nc.vector.tensor_reduce(ssb, scr, axis=mybir.AxisListType.X,
                                op=mybir.AluOpType.add)
        nc.sync.dma_start(out=scores[:, :], in_=ssb)
    nc.compile()
    return nc


def build_phase_b():
    """One biLSTM layer step (used for both layers; SPMD over 2 cores =
    fwd/bwd direction). Input x is the host-packed [128, 4, K, L] bf16
    transposed layout; output is the time-major bf16 history [128, L, BK]."""
    import concourse.bacc as bacc
    import concourse.mybir as mybir
    from concourse.tile import TileContext

    dt = mybir.dt
    AF = mybir.ActivationFunctionType
    f32, bf = dt.float32, dt.bfloat16
    nc = bacc.Bacc("TRN2", target_bir_lowering=False, debug=False, num_devices=2)
    kc_in = 4

    wih = nc.dram_tensor("wih", [D, G4], bf, kind="ExternalInput")
    whh = nc.dram_tensor("whh", [H, G4], bf, kind="ExternalInput")
    bias = nc.dram_tensor("bias", [G4], f32, kind="ExternalInput")
    x = nc.dram_tensor("x", [128, kc_in, K, L], bf, kind="ExternalInput")
    hout = nc.dram_tensor("hout", [128, L, BK], bf, kind="ExternalOutput")

    id_bf = nc.inline_tensor(np.eye(128, dtype=BF16), "idbf")

    from contextlib import ExitStack
    with TileContext(nc) as tc, ExitStack() as ectx:
        const = ectx.enter_context(tc.tile_pool(name="const", bufs=1))

        # warm the ACT tables (sigmoid/tanh ~2.6us each) under the weight DMAs
        warm = const.tile([128, 1], f32)
        nc.vector.memset(warm, 0.0)
        nc.scalar.activation(warm, warm, AF.Sigmoid)
        nc.scalar.activation(warm, warm, AF.Tanh)

        ident_bf = const.tile([128, 128], bf)
        nc.gpsimd.dma_start(out=ident_bf, in_=id_bf[:, :])
        wih_sb = const.tile([128, kc_in, G4], bf)
        nc.sync.dma_start(out=wih_sb[:, 0:2, :],
                          in_=wih.rearrange("(kc p) g -> p kc g", p=128)[:, 0:2, :])
        nc.scalar.dma_start(out=wih_sb[:, 2:4, :],
                            in_=wih.rearrange("(kc p) g -> p kc g", p=128)[:, 2:4, :])
        whh_sb = const.tile([128, NHC, G4], bf)
        nc.gpsimd.dma_start(out=whh_sb, in_=whh.rearrange("(hc p) g -> p hc g", p=128))
        bias_sb = const.tile([128, NGC], f32)
        nc.gpsimd.dma_start(out=bias_sb, in_=bias.rearrange("(gc p) -> p gc", p=128))
        xT = const.tile([128, kc_in, K, L], bf)
        nc.sync.dma_start(out=xT[:, 0:2, :, :], in_=x[:, 0:2, :, :])
        nc.scalar.dma_start(out=xT[:, 2:4, :, :], in_=x[:, 2:4, :, :])

        # ---- pre-projection: preT[:, gc, :, :] = wih[:, gc-cols].T @ xT + b
        preT = const.tile([128, NGC, K, L], bf)
        with ExitStack() as pctx:
            pre_ps = pctx.enter_context(tc.tile_pool(name="pre_ps", bufs=2,
                                                     space="PSUM"))
            for gc in range(NGC):
                psA = pre_ps.tile([128, 512], f32, tag="preA")
                psB = pre_ps.tile([128, 128], f32, tag="preB")
                for kc in range(kc_in):
                    lhsT = wih_sb[:, kc, 128 * gc:128 * (gc + 1)]
                    nc.tensor.matmul(psA, lhsT, xT[:, kc, 0:4, :],
                                     start=(kc == 0), stop=(kc == kc_in - 1))
                    nc.tensor.matmul(psB, lhsT, xT[:, kc, 4, :],
                                     start=(kc == 0), stop=(kc == kc_in - 1))
                bb = bias_sb[:, gc:gc + 1]
                nc.vector.tensor_add(preT[:, gc, 0:4, :], psA,
                                     bb.to_broadcast([128, 512]))
                nc.vector.tensor_add(preT[:, gc, 4, :], psB,
                                     bb.to_broadcast([128, 128]))

        # ---- 128-step scan
        step_ps = ectx.enter_context(tc.tile_pool(name="step_ps", bufs=3,
                                                  space="PSUM"))
        step_sb = ectx.enter_context(tc.tile_pool(name="step_sb", bufs=3))

        hbufT = const.tile([128, L + 1, BK], bf)   # time-major history
        tc_st = const.tile([128, 2 * BK], f32)     # [tg | c] cell state
        nc.vector.memset(hbufT[:, 0, :], 0.0)
        nc.vector.memset(tc_st[:, BK:2 * BK], 0.0)

        for t in range(L):
            ps_ifo = step_ps.tile([128, 6 * K], f32, tag="ifo", bufs=3)
            ps_g = step_ps.tile([128, 2 * K], f32, tag="g", bufs=3)
            # pre-gate injection: no dependence on h, so the PE executes these
            # during the previous step's elementwise tail.
            nc.tensor.matmul(ps_ifo, ident_bf, preT[:, 0:6, :, t],
                             start=True, stop=False)
            nc.tensor.matmul(ps_g, ident_bf, preT[:, 6:8, :, t],
                             start=True, stop=False)

            def h_rhs(hc):
                return hbufT[:, t, K * hc:K * (hc + 1)]

            # matmul order: g (tanh runs under the rest), then i/f (the
            # sigmoid the chain waits on), then o (its sigmoid is off-chain).
            for gc in (6, 7):
                for hc in range(NHC):
                    nc.tensor.matmul(
                        ps_g[:, K * (gc - 6):K * (gc - 5)],
                        whh_sb[:, hc, 128 * gc:128 * (gc + 1)], h_rhs(hc),
                        start=False, stop=(gc == 7 and hc == NHC - 1))
            for gc in (0, 1, 2, 3):
                for hc in range(NHC):
                    nc.tensor.matmul(
                        ps_ifo[:, K * gc:K * (gc + 1)],
                        whh_sb[:, hc, 128 * gc:128 * (gc + 1)], h_rhs(hc),
                        start=False, stop=(gc == 3 and hc == NHC - 1))
            for gc in (4, 5):
                for hc in range(NHC):
                    nc.tensor.matmul(
                        ps_ifo[:, K * gc:K * (gc + 1)],
                        whh_sb[:, hc, 128 * gc:128 * (gc + 1)], h_rhs(hc),
                        start=False, stop=(gc == 5 and hc == NHC - 1))

            # tanh(g) lands in tc_st[0:BK] while the i/f/o matmuls still run
            nc.scalar.activation(tc_st[:, 0:BK], ps_g, AF.Tanh)
            sg = step_sb.tile([128, 3 * BK], f32, tag="sg", bufs=3)
            nc.scalar.activation(sg[:, 0:2 * BK], ps_ifo[:, 0:2 * BK], AF.Sigmoid)
            nc.scalar.activation(sg[:, 2 * BK:3 * BK], ps_ifo[:, 2 * BK:3 * BK],
                                 AF.Sigmoid)
            zw = step_sb.tile([128, 2 * BK], f32, tag="zw", bufs=3)
            nc.vector.tensor_mul(zw, sg[:, 0:2 * BK], tc_st)   # [si*tg | sf*c]
            nc.vector.tensor_add(tc_st[:, BK:2 * BK], zw[:, 0:BK],
                                 zw[:, BK:2 * BK])             # c'
            th = step_sb.tile([128, BK], f32, tag="th", bufs=3)
            nc.scalar.activation(th, tc_st[:, BK:2 * BK], AF.Tanh)
            nc.vector.tensor_mul(hbufT[:, t + 1, :], sg[:, 2 * BK:3 * BK], th)

        # stream the history out in quarters so only the last 32 steps' DMA
        # trails the scan
        for q in range(4):
            nc.sync.dma_start(out=hout[:, 32 * q:32 * (q + 1), :],
                              in_=hbufT[:, 1 + 32 * q:1 + 32 * (q + 1), :])
    nc.compile()
    return nc


def build_phase_b3():
    import concourse.bacc as bacc
    import concourse.mybir as mybir
    from concourse.tile import TileContext

    dt = mybir.dt
    AO = mybir.AluOpType
    AF = mybir.ActivationFunctionType
    f32 = dt.float32
    nc = bacc.Bacc("TRN2", target_bir_lowering=False, debug=False, num_devices=1)

    bf = dt.bfloat16
    h1f = nc.dram_tensor("h1f", [128, NHC, K, L], bf, kind="ExternalInput")
    h1b = nc.dram_tensor("h1b", [128, NHC, K, L], bf, kind="ExternalInput")
    cs = nc.dram_tensor("cs", [D], f32, kind="ExternalInput")
    ctx_out = nc.dram_tensor("ctx", [K, D], f32, kind="ExternalOutput")
    id_f32 = nc.inline_tensor(np.eye(128, dtype=np.float32), "idf")

    DC = D // 128  # 4 chunks
    from contextlib import ExitStack
    with TileContext(nc) as tc, ExitStack() as ectx:
        pool = ectx.enter_context(tc.tile_pool(name="sb", bufs=1))
        ps_p = ectx.enter_context(tc.tile_pool(name="ps", bufs=2, space="PSUM"))
        sc_p = ectx.enter_context(tc.tile_pool(name="scratch", bufs=2))

        # warm the exp table under the input DMAs
        warm = pool.tile([128, 1], f32)
        nc.vector.memset(warm, 0.0)
        nc.scalar.activation(warm, warm, AF.Exp)

        lout = pool.tile([128, DC, K, L], bf)
        nc.sync.dma_start(out=lout[:, 0:NHC, :, :], in_=h1f[:, :, :, :])
        nc.scalar.dma_start(out=lout[:, NHC:DC, :, :], in_=h1b[:, :, :, :])
        cs_f = pool.tile([128, DC], f32)
        nc.gpsimd.dma_start(out=cs_f, in_=cs.rearrange("(kc p) -> p kc", p=128))
        cs_sb = pool.tile([128, DC], bf)
        nc.vector.tensor_copy(cs_sb, cs_f)
        ident_f = pool.tile([128, 128], f32)
        nc.gpsimd.dma_start(out=ident_f, in_=id_f32[:, :])

        # stationary = cs column broadcast to 128 identical columns -> every
        # out partition carries the same score row (free partition-broadcast)
        psA = ps_p.tile([128, 512], f32, tag="attA")
        psB = ps_p.tile([128, 128], f32, tag="attB")
        for kc in range(DC):
            csb_rep = cs_sb[:, kc:kc + 1].to_broadcast([128, 128])
            nc.tensor.matmul(psA, csb_rep, lout[:, kc, 0:4, :],
                             start=(kc == 0), stop=(kc == DC - 1))
            nc.tensor.matmul(psB, csb_rep, lout[:, kc, 4, :],
                             start=(kc == 0), stop=(kc == DC - 1))
        esb = pool.tile([128, K, L], f32)
        nc.scalar.activation(esb[:, 0:4, :], psA, AF.Exp)
        nc.scalar.activation(esb[:, 4, :], psB, AF.Exp)
        se = pool.tile([128, K], f32)
        nc.vector.tensor_reduce(se, esb, axis=mybir.AxisListType.X, op=AO.add)
        rse = pool.tile([128, K], f32)
        nc.vector.reciprocal(rse, se)
        attw = pool.tile([128, K, L], f32)
        nc.vector.tensor_mul(attw, esb, rse.unsqueeze(2).to_broadcast([128, K, L]))

        ctxT = pool.tile([128, DC, K], f32)
        wsc = sc_p.tile([128, DC, K, L], f32, tag="wsc")
        nc.vector.tensor_mul(wsc, lout,
                             attw.unsqueeze(1).to_broadcast([128, DC, K, L]))
        nc.vector.tensor_reduce(ctxT, wsc, axis=mybir.AxisListType.X, op=AO.add)
        csb = pool.tile([K, DC, 128], f32)
        for kc in range(DC):
            pst = ps_p.tile([K, 128], f32, tag="tp")
            nc.tensor.transpose(pst, ctxT[:, kc, :], ident_f)
            nc.vector.tensor_copy(csb[:, kc, :], pst)
        nc.sync.dma_start(out=ctx_out[:, :], in_=csb)
    nc.compile()
    return nc


# --------------------------------------------------------------------------
# host-side weight prep
# --------------------------------------------------------------------------

def _prep_lstm_weights(w_ih, w_hh, b_ih, b_hh, perm_input_halves=False):
    def reorder(m):
        # torch gate order [i, f, g, o] -> kernel order [i, f, o, g]
        i, f, g, o = np.split(m, 4, axis=0)
        return np.concatenate([i, f, o, g], axis=0)

    wihT = np.ascontiguousarray(reorder(np.asarray(w_ih, np.float32)).T)
    whhT = np.ascontiguousarray(reorder(np.asarray(w_hh, np.float32)).T)
    bias = reorder((np.asarray(b_ih, np.float32) + np.asarray(b_hh, np.float32))[:, None])[:, 0]
    if perm_input_halves:
        wihT = np.concatenate([wihT[H:2 * H], wihT[0:H]], axis=0)
    return (np.ascontiguousarray(wihT.astype(BF16)),
            np.ascontiguousarray(whhT.astype(BF16)),
            np.ascontiguousarray(bias.astype(np.float32)))


def _get(name, builder):
    if name not in _cache:
        _cache[name] = builder()
    return _cache[name]


def _ensure_ntff_hook():
    """The image's antenv lacks axon_hooks; synthesize it and register the
    ctypes NTFF profiling hook from trn_agent_boot so trace=True works."""
    import sys
    import types
    try:
        from antenv.axon_hooks import get_axon_ntff_profile_hook  # noqa: F401
        return
    except ImportError:
        pass
    import antenv
    mod = types.ModuleType("antenv.axon_hooks")
    mod._hook = None

    def set_axon_ntff_profile_hook(h):
        mod._hook = h

    def get_axon_ntff_profile_hook():
        return mod._hook

    mod.set_axon_ntff_profile_hook = set_axon_ntff_profile_hook
    mod.get_axon_ntff_profile_hook = get_axon_ntff_profile_hook
    sys.modules["antenv.axon_hooks"] = mod
    antenv.axon_hooks = mod
    try:
        from trn_agent_boot.trn_boot import _ntff_profile_via_ctypes
        hook = _ntff_profile_via_ctypes('/opt/axon/libaxon_pjrt.so')
        if hook is not None:
            mod._hook = hook
    except Exception:
        pass


def _run(nc, in_maps, core_ids, trace=False):
    from concourse.bass_utils import run_bass_kernel_spmd
    if trace:
        try:
            _ensure_ntff_hook()
            return run_bass_kernel_spmd(nc, in_maps, core_ids, trace=True)
        except Exception as e:
            print(f"trace run failed ({type(e).__name__}: {e}); retrying untraced")
    return run_bass_kernel_spmd(nc, in_maps, core_ids, trace=False)


# --------------------------------------------------------------------------
# main entry
# --------------------------------------------------------------------------

def _pack_xT_from_h(hT, flip=False):
    """[128, L, BK] bf16 scan history -> [128, NHC, K, L] input chunk."""
    a = np.asarray(hT)
    if flip:
        a = a[:, ::-1, :]
    return np.transpose(a.reshape(128, L, NHC, K), (0, 2, 3, 1))


def kernel(episodes, query, current_state, ages, Wq, bq, Wk, bk,
           w_ih_l0, w_hh_l0, b_ih_l0, b_hh_l0,
           w_ih_l0r, w_hh_l0r, b_ih_l0r, b_hh_l0r,
           w_ih_l1, w_hh_l1, b_ih_l1, b_hh_l1,
           w_ih_l1r, w_hh_l1r, b_ih_l1r, b_hh_l1r, k,
           _collect_times=None):
    episodes = np.asarray(episodes, np.float32)
    query = np.asarray(query, np.float32)
    current_state = np.asarray(current_state, np.float32)
    ages = np.asarray(ages, np.float32)
    assert int(k) == K

    times = _collect_times if _collect_times is not None else None
    trace = times is not None

    def note(res):
        if times is not None:
            times.append(res.exec_time_ns)

    # ---- phase A
    qp = np.asarray(Wq, np.float32) @ query + np.asarray(bq, np.float32)
    v = (np.asarray(Wk, np.float32).T @ qp) / np.float32(L)
    vrep = np.ascontiguousarray(np.broadcast_to(v, (128, D)), dtype=np.float32)
    nc_a = _get("A", build_phase_a)
    in_maps = [{"ep": episodes[c * EPC:(c + 1) * EPC], "vrep": vrep}
               for c in range(NC)]
    res = _run(nc_a, in_maps, list(range(NC)), trace)
    note(res)
    scores = np.concatenate([res.results[c]["scores"][:, 0] for c in range(NC)])

    idx = np.argsort(-scores, kind="stable")[:K]
    w_rec = (1.0 / (1.0 + ages[idx] * np.float32(0.01))).astype(np.float32)
    xsel = episodes[idx] * w_rec[:, None, None]

    # ---- phase B1 (layer 0)
    wi0, wh0, b0 = _prep_lstm_weights(w_ih_l0, w_hh_l0, b_ih_l0, b_hh_l0)
    wi0r, wh0r, b0r = _prep_lstm_weights(w_ih_l0r, w_hh_l0r, b_ih_l0r, b_hh_l0r)
    nc_b = _get("B", build_phase_b)

    def to_xT(xs):  # [5, 128, 512] f32 -> [128, 4, 5, 128] bf16
        xT = np.transpose(xs, (2, 0, 1)).reshape(4, 128, K, L)
        return np.ascontiguousarray(np.transpose(xT, (1, 0, 2, 3)).astype(BF16))

    in_maps = [
        {"x": to_xT(xsel), "wih": wi0, "whh": wh0, "bias": b0},
        {"x": to_xT(xsel[:, ::-1, :]), "wih": wi0r, "whh": wh0r, "bias": b0r},
    ]
    res = _run(nc_b, in_maps, [0, 1], trace)
    note(res)
    h0_c0 = np.asarray(res.results[0]["hout"])  # bf16 [128, L, BK]
    h0_c1 = np.asarray(res.results[1]["hout"])

    # ---- phase B2 (layer 1)
    wi1, wh1, b1 = _prep_lstm_weights(w_ih_l1, w_hh_l1, b_ih_l1, b_hh_l1)
    wi1r, wh1r, b1r = _prep_lstm_weights(w_ih_l1r, w_hh_l1r, b_ih_l1r, b_hh_l1r,
                                         perm_input_halves=True)
    x_c0 = np.ascontiguousarray(np.concatenate(
        [_pack_xT_from_h(h0_c0), _pack_xT_from_h(h0_c1, flip=True)], axis=1))
    x_c1 = np.ascontiguousarray(np.concatenate(
        [_pack_xT_from_h(h0_c1), _pack_xT_from_h(h0_c0, flip=True)], axis=1))
    in_maps = [
        {"x": x_c0, "wih": wi1, "whh": wh1, "bias": b1},
        {"x": x_c1, "wih": wi1r, "whh": wh1r, "bias": b1r},
    ]
    res = _run(nc_b, in_maps, [0, 1], trace)
    note(res)
    h1_c0 = np.asarray(res.results[0]["hout"])  # bf16 [128, L, BK]
    h1_c1 = np.asarray(res.results[1]["hout"])

    # ---- phase B3 (attention)
    nc_b3 = _get("B3", build_phase_b3)
    in_maps = [{"h1f": np.ascontiguousarray(_pack_xT_from_h(h1_c0)),
                "h1b": np.ascontiguousarray(_pack_xT_from_h(h1_c1, flip=True)),
                "cs": current_state}]
    res = _run(nc_b3, in_maps, [0], trace)
    note(res)
    ctx = np.asarray(res.results[0]["ctx"], np.float32)  # [5, 512]
    return ctx[:, None, :]


# revision 14
# speedup vs baseline: 1.2074x; 1.0264x over previous
"""Trainium2 Bass kernel for nn_EpisodicMemory (retrieval_knn).

Pipeline (4 SPMD launches, all compiled once per process and cached):
  A  (8 cores): episode-bank scoring. Each core owns 128 episodes [128,128,512]
     loaded episode-major ([128 ep-partitions, 16 L, 512 D] tiles, 32KB
     contiguous per partition -> line-rate DMA descriptors). The L-sum is a
     DVE in-place tree-add per tile (hidden under the DMA), then one
     mul+reduce against the host-precomputed v = Wk.T @ (Wq q + bq) / L.
  host: stable top-k, recency prescale of the 5 selected episodes.
  B1 (2 cores): biLSTM layer 0, transposed layout (gate dim -> partitions,
     batch -> free). Per step: 2 pre-gate injection matmuls (identity x preT,
     emitted one step ahead so they fill the PE during the previous step's
     elementwise tail), 16 recurrent matmuls (g-gates first so tanh(g) runs
     under the i/f/o matmuls), one sigmoid over [i|f|o], fused [z|w] multiply
     against the SBUF-resident [tg|c] state, add, tanh, and a bf16 multiply
     that writes h directly into the time-major history buffer.
  B2 (2 cores): biLSTM layer 1 -- same compiled program; the host repacks the
     layer-0 histories (transpose + time flip) between launches.
  B3 (1 core): temporal attention over the 5 scanned episodes in fp32.
"""

import numpy as np
import ml_dtypes

BF16 = ml_dtypes.bfloat16

N, L, D, H = 1024, 128, 512, 256
K = 5
NC = 8
EPC = N // NC  # 128 episodes per core
G4 = 4 * H     # 1024 gate dims
NGC = G4 // 128  # 8 gate chunks
NHC = H // 128   # 2 hidden chunks
BK = NHC * K     # 10

_cache = {}


def _enable_fwl():
    # concourse pins --enable-ldw-opt=false; walrus rejects bass InstLdweights
    # under ldw-opt, so leave it off (FWL still engages for bf16 128-col
    # stationaries per the compiler's automatic EnableFWL path).
    pass


# --------------------------------------------------------------------------
# program builders
# --------------------------------------------------------------------------

def build_phase_a():
    import concourse.bacc as bacc
    import concourse.mybir as mybir
    from concourse.tile import TileContext

    dt = mybir.dt
    f32 = dt.float32
    nc = bacc.Bacc("TRN2", target_bir_lowering=False, debug=False, num_devices=NC)
    ep = nc.dram_tensor("ep", [EPC, L, D], f32, kind="ExternalInput")
    vrep = nc.dram_tensor("vrep", [128, D], f32, kind="ExternalInput")
    scores = nc.dram_tensor("scores", [EPC, 1], f32, kind="ExternalOutput")

    LC = 8  # L rows per DMA tile: [128, 8, 512] f32 = 16KB/partition contiguous

    from contextlib import ExitStack
    with TileContext(nc) as tc, ExitStack() as ectx:
        const = ectx.enter_context(tc.tile_pool(name="const", bufs=1))
        dma_p = ectx.enter_context(tc.tile_pool(name="eps", bufs=10))

        # all episode DMAs on ONE HWDGE ring: the SDMA engines drain the ring
        # FIFO, so tile 0 completes first instead of all tiles progressing in
        # lockstep (which delays the first reduction by ~20us). Deep buffering
        # (10 tiles) decouples the DMA stream from the DVE reduction.
        tiles = []
        for i in range(L // LC):
            t = dma_p.tile([128, LC, D], f32, tag="ep")
            nc.sync.dma_start(out=t, in_=ep[:, LC * i:LC * (i + 1), :])
            tiles.append(t)
        vrep_sb = const.tile([128, D], f32)
        nc.scalar.dma_start(out=vrep_sb, in_=vrep[:, :])
        acc = const.tile([128, D], f32)
        nc.vector.memset(acc, 0.0)

        for i in range(L // LC):
            t = tiles[i]
            # in-place tree reduction over the 8 L-slabs
            nc.vector.tensor_add(t[:, 0:4, :], t[:, 0:4, :], t[:, 4:8, :])
            nc.vector.tensor_add(t[:, 0:2, :], t[:, 0:2, :], t[:, 2:4, :])
            nc.vector.tensor_add(t[:, 0, :], t[:, 0, :], t[:, 1, :])
            nc.vector.tensor_add(acc, acc, t[:, 0, :])

        scr = const.tile([128, D], f32)
        nc.vector.tensor_mul(scr, acc, vrep_sb)
        ssb = const.tile([128, 1], f32)
        nc.vector.tensor_reduce(ssb, scr, axis=mybir.AxisListType.X,
                                op=mybir.AluOpType.add)
        nc.sync.dma_start(out=scores[:, :], in_=ssb)
    nc.compile()
    return nc


def build_phase_b():
    """One biLSTM layer step (used for both layers; SPMD over 2 cores =
    fwd/bwd direction). Input x is the host-packed [128, 4, K, L] bf16
    transposed layout; output is the time-major bf16 history [128, L, BK]."""
    import concourse.bacc as bacc
    import concourse.mybir as mybir
    from concourse.tile import TileContext

    dt = mybir.dt
    AF = mybir.ActivationFunctionType
    f32, bf = dt.float32, dt.bfloat16
    nc = bacc.Bacc("TRN2", target_bir_lowering=False, debug=False, num_devices=2)
    kc_in = 4

    wih = nc.dram_tensor("wih", [D, G4], bf, kind="ExternalInput")
    whh = nc.dram_tensor("whh", [H, G4], bf, kind="ExternalInput")
    bias = nc.dram_tensor("bias", [G4], f32, kind="ExternalInput")
    x = nc.dram_tensor("x", [128, kc_in, K, L], bf, kind="ExternalInput")
    hout = nc.dram_tensor("hout", [128, L, BK], bf, kind="ExternalOutput")

    id_bf = nc.inline_tensor(np.eye(128, dtype=BF16), "idbf")

    from contextlib import ExitStack
    with TileContext(nc) as tc, ExitStack() as ectx:
        const = ectx.enter_context(tc.tile_pool(name="const", bufs=1))

        # warm the ACT tables (sigmoid/tanh ~2.6us each) under the weight DMAs
        warm = const.tile([128, 1], f32)
        nc.vector.memset(warm, 0.0)
        nc.scalar.activation(warm, warm, AF.Sigmoid)
        nc.scalar.activation(warm, warm, AF.Tanh)

        ident_bf = const.tile([128, 128], bf)
        nc.gpsimd.dma_start(out=ident_bf, in_=id_bf[:, :])
        wih_sb = const.tile([128, kc_in, G4], bf)
        nc.sync.dma_start(out=wih_sb[:, 0:2, :],
                          in_=wih.rearrange("(kc p) g -> p kc g", p=128)[:, 0:2, :])
        nc.scalar.dma_start(out=wih_sb[:, 2:4, :],
                            in_=wih.rearrange("(kc p) g -> p kc g", p=128)[:, 2:4, :])
        whh_sb = const.tile([128, NHC, G4], bf)
        nc.gpsimd.dma_start(out=whh_sb, in_=whh.rearrange("(hc p) g -> p hc g", p=128))
        bias_sb = const.tile([128, NGC], f32)
        nc.gpsimd.dma_start(out=bias_sb, in_=bias.rearrange("(gc p) -> p gc", p=128))
        xT = const.tile([128, kc_in, K, L], bf)
        nc.sync.dma_start(out=xT[:, 0:2, :, :], in_=x[:, 0:2, :, :])
        nc.scalar.dma_start(out=xT[:, 2:4, :, :], in_=x[:, 2:4, :, :])

        # warm the PE HAM clock gate during the weight DMAs with 1-column
        # matmuls: enough sustained activity to reach K=8/8 before the
        # pre-projection, but ~1/128 of full-array power (dense full-array
        # warmups trip the P0 power-state downclock chip-wide).
        with ExitStack() as wctx:
            warm_ps = wctx.enter_context(tc.tile_pool(name="warm_ps", bufs=1,
                                                      space="PSUM"))
            wps = warm_ps.tile([128, 1], f32)
            for _ in range(96):
                nc.tensor.matmul(wps[0:1, :], ident_bf[:, 0:1],
                                 ident_bf[:, 0:1], start=True, stop=True)

        # ---- pre-projection: preT[:, gc, :, :] = wih[:, gc-cols].T @ xT + b
        preT = const.tile([128, NGC, K, L], bf)
        with ExitStack() as pctx:
            pre_ps = pctx.enter_context(tc.tile_pool(name="pre_ps", bufs=2,
                                                     space="PSUM"))
            for gc in range(NGC):
                psA = pre_ps.tile([128, 512], f32, tag="preA")
                psB = pre_ps.tile([128, 128], f32, tag="preB")
                for kc in range(kc_in):
                    lhsT = wih_sb[:, kc, 128 * gc:128 * (gc + 1)]
                    nc.tensor.matmul(psA, lhsT, xT[:, kc, 0:4, :],
                                     start=(kc == 0), stop=(kc == kc_in - 1))
                    nc.tensor.matmul(psB, lhsT, xT[:, kc, 4, :],
                                     start=(kc == 0), stop=(kc == kc_in - 1))
                bb = bias_sb[:, gc:gc + 1]
                nc.vector.tensor_add(preT[:, gc, 0:4, :], psA,
                                     bb.to_broadcast([128, 512]))
                nc.vector.tensor_add(preT[:, gc, 4, :], psB,
                                     bb.to_broadcast([128, 128]))

        # ---- 128-step scan
        step_ps = ectx.enter_context(tc.tile_pool(name="step_ps", bufs=3,
                                                  space="PSUM"))
        step_sb = ectx.enter_context(tc.tile_pool(name="step_sb", bufs=3))

        hbufT = const.tile([128, L + 1, BK], bf)   # time-major history
        tc_st = const.tile([128, 2 * BK], f32)     # [tg | c] cell state
        nc.vector.memset(hbufT[:, 0, :], 0.0)
        nc.vector.memset(tc_st[:, BK:2 * BK], 0.0)

        for t in range(L):
            ps_ifo = step_ps.tile([128, 6 * K], f32, tag="ifo", bufs=3)
            ps_g = step_ps.tile([128, 2 * K], f32, tag="g", bufs=3)
            # pre-gate injection: no dependence on h, so the PE executes these
            # during the previous step's elementwise tail.
            nc.tensor.matmul(ps_ifo, ident_bf, preT[:, 0:6, :, t],
                             start=True, stop=False)
            nc.tensor.matmul(ps_g, ident_bf, preT[:, 6:8, :, t],
                             start=True, stop=False)

            def h_rhs(hc):
                return hbufT[:, t, K * hc:K * (hc + 1)]

            # matmul order: g (tanh runs under the rest), then i/f (the
            # sigmoid the chain waits on), then o (its sigmoid is off-chain).
            for gc in (6, 7):
                for hc in range(NHC):
                    nc.tensor.matmul(
                        ps_g[:, K * (gc - 6):K * (gc - 5)],
                        whh_sb[:, hc, 128 * gc:128 * (gc + 1)], h_rhs(hc),
                        start=False, stop=(gc == 7 and hc == NHC - 1))
            for gc in (0, 1, 2, 3):
                for hc in range(NHC):
                    nc.tensor.matmul(
                        ps_ifo[:, K * gc:K * (gc + 1)],
                        whh_sb[:, hc, 128 * gc:128 * (gc + 1)], h_rhs(hc),
                        start=False, stop=(gc == 3 and hc == NHC - 1))
            for gc in (4, 5):
                for hc in range(NHC):
                    nc.tensor.matmul(
                        ps_ifo[:, K * gc:K * (gc + 1)],
                        whh_sb[:, hc, 128 * gc:128 * (gc + 1)], h_rhs(hc),
                        start=False, stop=(gc == 5 and hc == NHC - 1))

            # tanh(g) lands in tc_st[0:BK] while the i/f/o matmuls still run
            nc.scalar.activation(tc_st[:, 0:BK], ps_g, AF.Tanh)
            sg = step_sb.tile([128, 3 * BK], f32, tag="sg", bufs=3)
            nc.scalar.activation(sg[:, 0:2 * BK], ps_ifo[:, 0:2 * BK], AF.Sigmoid)
            nc.scalar.activation(sg[:, 2 * BK:3 * BK], ps_ifo[:, 2 * BK:3 * BK],
                                 AF.Sigmoid)
            zw = step_sb.tile([128, 2 * BK], f32, tag="zw", bufs=3)
            nc.vector.tensor_mul(zw, sg[:, 0:2 * BK], tc_st)   # [si*tg | sf*c]
            nc.vector.tensor_add(tc_st[:, BK:2 * BK], zw[:, 0:BK],
                                 zw[:, BK:2 * BK])             # c'
            th = step_sb.tile([128, BK], f32, tag="th", bufs=3)
            nc.scalar.activation(th, tc_st[:, BK:2 * BK], AF.Tanh)
            nc.vector.tensor_mul(hbufT[:, t + 1, :], sg[:, 2 * BK:3 * BK], th)

        # stream the history out in quarters so only the last 32 steps' DMA
        # trails the scan
        for q in range(4):
            nc.sync.dma_start(out=hout[:, 32 * q:32 * (q + 1), :],
                              in_=hbufT[:, 1 + 32 * q:1 + 32 * (q + 1), :])
    nc.compile()
    return nc


def build_phase_b3():
    import concourse.bacc as bacc
    import concourse.mybir as mybir
    from concourse.tile import TileContext

    dt = mybir.dt
    AO = mybir.AluOpType
    AF = mybir.ActivationFunctionType
    f32 = dt.float32
    nc = bacc.Bacc("TRN2", target_bir_lowering=False, debug=False, num_devices=1)

    bf = dt.bfloat16
    h1f = nc.dram_tensor("h1f", [128, NHC, K, L], bf, kind="ExternalInput")
    h1b = nc.dram_tensor("h1b", [128, NHC, K, L], bf, kind="ExternalInput")
    cs = nc.dram_tensor("cs", [D], f32, kind="ExternalInput")
    ctx_out = nc.dram_tensor("ctx", [K, D], f32, kind="ExternalOutput")
    id_f32 = nc.inline_tensor(np.eye(128, dtype=np.float32), "idf")

    DC = D // 128  # 4 chunks
    from contextlib import ExitStack
    with TileContext(nc) as tc, ExitStack() as ectx:
        pool = ectx.enter_context(tc.tile_pool(name="sb", bufs=1))
        ps_p = ectx.enter_context(tc.tile_pool(name="ps", bufs=2, space="PSUM"))
        sc_p = ectx.enter_context(tc.tile_pool(name="scratch", bufs=2))

        # warm the exp table under the input DMAs
        warm = pool.tile([128, 1], f32)
        nc.vector.memset(warm, 0.0)
        nc.scalar.activation(warm, warm, AF.Exp)

        lout = pool.tile([128, DC, K, L], bf)
        nc.sync.dma_start(out=lout[:, 0:NHC, :, :], in_=h1f[:, :, :, :])
        nc.scalar.dma_start(out=lout[:, NHC:DC, :, :], in_=h1b[:, :, :, :])
        cs_f = pool.tile([128, DC], f32)
        nc.gpsimd.dma_start(out=cs_f, in_=cs.rearrange("(kc p) -> p kc", p=128))
        cs_sb = pool.tile([128, DC], bf)
        nc.vector.tensor_copy(cs_sb, cs_f)
        ident_f = pool.tile([128, 128], f32)
        nc.gpsimd.dma_start(out=ident_f, in_=id_f32[:, :])

        # stationary = cs column broadcast to 128 identical columns -> every
        # out partition carries the same score row (free partition-broadcast)
        psA = ps_p.tile([128, 512], f32, tag="attA")
        psB = ps_p.tile([128, 128], f32, tag="attB")
        for kc in range(DC):
            csb_rep = cs_sb[:, kc:kc + 1].to_broadcast([128, 128])
            nc.tensor.matmul(psA, csb_rep, lout[:, kc, 0:4, :],
                             start=(kc == 0), stop=(kc == DC - 1))
            nc.tensor.matmul(psB, csb_rep, lout[:, kc, 4, :],
                             start=(kc == 0), stop=(kc == DC - 1))
        esb = pool.tile([128, K, L], f32)
        nc.scalar.activation(esb[:, 0:4, :], psA, AF.Exp)
        nc.scalar.activation(esb[:, 4, :], psB, AF.Exp)
        se = pool.tile([128, K], f32)
        nc.vector.tensor_reduce(se, esb, axis=mybir.AxisListType.X, op=AO.add)
        rse = pool.tile([128, K], f32)
        nc.vector.reciprocal(rse, se)
        attw = pool.tile([128, K, L], f32)
        nc.vector.tensor_mul(attw, esb, rse.unsqueeze(2).to_broadcast([128, K, L]))

        ctxT = pool.tile([128, DC, K], f32)
        wsc = sc_p.tile([128, DC, K, L], f32, tag="wsc")
        nc.vector.tensor_mul(wsc, lout,
                             attw.unsqueeze(1).to_broadcast([128, DC, K, L]))
        nc.vector.tensor_reduce(ctxT, wsc, axis=mybir.AxisListType.X, op=AO.add)
        csb = pool.tile([K, DC, 128], f32)
        for kc in range(DC):
            pst = ps_p.tile([K, 128], f32, tag="tp")
            nc.tensor.transpose(pst, ctxT[:, kc, :], ident_f)
            nc.vector.tensor_copy(csb[:, kc, :], pst)
        nc.sync.dma_start(out=ctx_out[:, :], in_=csb)
    nc.compile()
    return nc


# --------------------------------------------------------------------------
# host-side weight prep
# --------------------------------------------------------------------------

def _prep_lstm_weights(w_ih, w_hh, b_ih, b_hh, perm_input_halves=False):
    def reorder(m):
        # torch gate order [i, f, g, o] -> kernel order [i, f, o, g]
        i, f, g, o = np.split(m, 4, axis=0)
        return np.concatenate([i, f, o, g], axis=0)

    wihT = np.ascontiguousarray(reorder(np.asarray(w_ih, np.float32)).T)
    whhT = np.ascontiguousarray(reorder(np.asarray(w_hh, np.float32)).T)
    bias = reorder((np.asarray(b_ih, np.float32) + np.asarray(b_hh, np.float32))[:, None])[:, 0]
    if perm_input_halves:
        wihT = np.concatenate([wihT[H:2 * H], wihT[0:H]], axis=0)
    return (np.ascontiguousarray(wihT.astype(BF16)),
            np.ascontiguousarray(whhT.astype(BF16)),
            np.ascontiguousarray(bias.astype(np.float32)))


def _get(name, builder):
    if name not in _cache:
        _cache[name] = builder()
    return _cache[name]


def _ensure_ntff_hook():
    """The image's antenv lacks axon_hooks; synthesize it and register the
    ctypes NTFF profiling hook from trn_agent_boot so trace=True works."""
    import sys
    import types
    try:
        from antenv.axon_hooks import get_axon_ntff_profile_hook  # noqa: F401
        return
    except ImportError:
        pass
    import antenv
    mod = types.ModuleType("antenv.axon_hooks")
    mod._hook = None

    def set_axon_ntff_profile_hook(h):
        mod._hook = h

    def get_axon_ntff_profile_hook():
        return mod._hook

    mod.set_axon_ntff_profile_hook = set_axon_ntff_profile_hook
    mod.get_axon_ntff_profile_hook = get_axon_ntff_profile_hook
    sys.modules["antenv.axon_hooks"] = mod
    antenv.axon_hooks = mod
    try:
        from trn_agent_boot.trn_boot import _ntff_profile_via_ctypes
        hook = _ntff_profile_via_ctypes('/opt/axon/libaxon_pjrt.so')
        if hook is not None:
            mod._hook = hook
    except Exception:
        pass


def _run(nc, in_maps, core_ids, trace=False):
    from concourse.bass_utils import run_bass_kernel_spmd
    if trace:
        try:
            _ensure_ntff_hook()
            return run_bass_kernel_spmd(nc, in_maps, core_ids, trace=True)
        except Exception as e:
            print(f"trace run failed ({type(e).__name__}: {e}); retrying untraced")
    return run_bass_kernel_spmd(nc, in_maps, core_ids, trace=False)


# --------------------------------------------------------------------------
# main entry
# --------------------------------------------------------------------------

def _pack_xT_from_h(hT, flip=False):
    """[128, L, BK] bf16 scan history -> [128, NHC, K, L] input chunk."""
    a = np.asarray(hT)
    if flip:
        a = a[:, ::-1, :]
    return np.transpose(a.reshape(128, L, NHC, K), (0, 2, 3, 1))


def kernel(episodes, query, current_state, ages, Wq, bq, Wk, bk,
           w_ih_l0, w_hh_l0, b_ih_l0, b_hh_l0,
           w_ih_l0r, w_hh_l0r, b_ih_l0r, b_hh_l0r,
           w_ih_l1, w_hh_l1, b_ih_l1, b_hh_l1,
           w_ih_l1r, w_hh_l1r, b_ih_l1r, b_hh_l1r, k,
           _collect_times=None):
    episodes = np.asarray(episodes, np.float32)
    query = np.asarray(query, np.float32)
    current_state = np.asarray(current_state, np.float32)
    ages = np.asarray(ages, np.float32)
    assert int(k) == K

    times = _collect_times if _collect_times is not None else None
    trace = times is not None

    def note(res):
        if times is not None:
            times.append(res.exec_time_ns)

    # ---- phase A
    qp = np.asarray(Wq, np.float32) @ query + np.asarray(bq, np.float32)
    v = (np.asarray(Wk, np.float32).T @ qp) / np.float32(L)
    vrep = np.ascontiguousarray(np.broadcast_to(v, (128, D)), dtype=np.float32)
    nc_a = _get("A", build_phase_a)
    in_maps = [{"ep": episodes[c * EPC:(c + 1) * EPC], "vrep": vrep}
               for c in range(NC)]
    res = _run(nc_a, in_maps, list(range(NC)), trace)
    note(res)
    scores = np.concatenate([res.results[c]["scores"][:, 0] for c in range(NC)])

    idx = np.argsort(-scores, kind="stable")[:K]
    w_rec = (1.0 / (1.0 + ages[idx] * np.float32(0.01))).astype(np.float32)
    xsel = episodes[idx] * w_rec[:, None, None]

    # ---- phase B1 (layer 0)
    wi0, wh0, b0 = _prep_lstm_weights(w_ih_l0, w_hh_l0, b_ih_l0, b_hh_l0)
    wi0r, wh0r, b0r = _prep_lstm_weights(w_ih_l0r, w_hh_l0r, b_ih_l0r, b_hh_l0r)
    nc_b = _get("B", build_phase_b)

    def to_xT(xs):  # [5, 128, 512] f32 -> [128, 4, 5, 128] bf16
        xT = np.transpose(xs, (2, 0, 1)).reshape(4, 128, K, L)
        return np.ascontiguousarray(np.transpose(xT, (1, 0, 2, 3)).astype(BF16))

    in_maps = [
        {"x": to_xT(xsel), "wih": wi0, "whh": wh0, "bias": b0},
        {"x": to_xT(xsel[:, ::-1, :]), "wih": wi0r, "whh": wh0r, "bias": b0r},
    ]
    res = _run(nc_b, in_maps, [0, 1], trace)
    note(res)
    h0_c0 = np.asarray(res.results[0]["hout"])  # bf16 [128, L, BK]
    h0_c1 = np.asarray(res.results[1]["hout"])

    # ---- phase B2 (layer 1)
    wi1, wh1, b1 = _prep_lstm_weights(w_ih_l1, w_hh_l1, b_ih_l1, b_hh_l1)
    wi1r, wh1r, b1r = _prep_lstm_weights(w_ih_l1r, w_hh_l1r, b_ih_l1r, b_hh_l1r,
                                         perm_input_halves=True)
    x_c0 = np.ascontiguousarray(np.concatenate(
        [_pack_xT_from_h(h0_c0), _pack_xT_from_h(h0_c1, flip=True)], axis=1))
    x_c1 = np.ascontiguousarray(np.concatenate(
        [_pack_xT_from_h(h0_c1), _pack_xT_from_h(h0_c0, flip=True)], axis=1))
    in_maps = [
        {"x": x_c0, "wih": wi1, "whh": wh1, "bias": b1},
        {"x": x_c1, "wih": wi1r, "whh": wh1r, "bias": b1r},
    ]
    res = _run(nc_b, in_maps, [0, 1], trace)
    note(res)
    h1_c0 = np.asarray(res.results[0]["hout"])  # bf16 [128, L, BK]
    h1_c1 = np.asarray(res.results[1]["hout"])

    # ---- phase B3 (attention)
    nc_b3 = _get("B3", build_phase_b3)
    in_maps = [{"h1f": np.ascontiguousarray(_pack_xT_from_h(h1_c0)),
                "h1b": np.ascontiguousarray(_pack_xT_from_h(h1_c1, flip=True)),
                "cs": current_state}]
    res = _run(nc_b3, in_maps, [0], trace)
    note(res)
    ctx = np.asarray(res.results[0]["ctx"], np.float32)  # [5, 512]
    return ctx[:, None, :]
